# revision 2
# baseline (speedup 1.0000x reference)
"""Trainium2 Bass kernel for nn_GatedJunction (gated multi-branch junction).

Math (per batch element b):
    m_y  = mean_hw(y[b])                     # [C]
    m_xk = mean_hw(x_k[b])                   # [C] for k=0..3
    feats = concat(m_y, m_x0..m_x3)          # [5C] = [1280]
    h  = relu(bn(feats @ conv1_w.T))         # [32]
    w  = h @ conv2_w.T + conv2_b             # [1280] -> [5, 256]
    w1 = sigmoid(w[0])                       # self gate  [256]
    w2 = softmax_k(w[1:])                    # branch gates [4, 256]
    out[b] = y[b]*w1[:,None,None] + sum_k w2[k][:,None,None]*x_k[b]

Sharding: data-parallel over batch. 8 cores x 4 batch elements each.
Params are tiny and replicated to every core.

The kernel is memory-bound (reads 5 maps, writes 1).  The feature maps are
staged to DRAM in fp16 with a partition-major host layout so each batch is
ONE contiguous-per-partition 2.5 MB DMA; the output is stored fp16 and
up-cast on host.  End-to-end fp16 error is ~1e-3 relative (gate is 2e-2).

On-core layout: channel-on-partition.  Each batch is one SBUF tile
[128, 5, 2048] (c%128 on partitions; tensor, c//128*1024 + hw on free).
Channel sums via accum_out on ACT/DVE; the gate MLP runs on PE with
conv2_b folded in as a 33rd contraction row; pass 2 is per-partition-scalar
FMAs (scalar_tensor_tensor) on DVE with the y*w1 init on ACT.  Loads are
emitted LOOKAHEAD batches ahead of compute so the SP DMA queue always has
work queued; stores follow compute on the same queue (their waits coincide
with the following load's buffer dependency, so they add no stalls).
"""

import sys

for _p in ("/root/.axon_site/_ro/trn_rl_repo", "/opt/trn_rl_repo"):
    if _p not in sys.path:
        sys.path.append(_p)

from contextlib import ExitStack

import numpy as np

import concourse.bass as bass
import concourse.tile as tile
from concourse import mybir
from concourse.bass_utils import run_bass_kernel_spmd

# Problem constants (hardcoded from the spec).
B, K, C, H, W = 32, 4, 256, 32, 32
MID = 32
EPS = 1e-5
HW = H * W          # 1024
N_CORES = 8
B_LOC = B // N_CORES  # 4
NT = K + 1          # 5 tensors: y, x0..x3
FEAT = NT * C       # 1280
NCH = FEAT // 128   # 10 feature chunks of 128
CH = C // 128       # 2 channel chunks per tensor
FD = CH * HW        # 2048 free-dim elems per (tensor, batch) per partition

FP32 = mybir.dt.float32
FP16 = mybir.dt.float16
ALU = mybir.AluOpType
AF = mybir.ActivationFunctionType

LOOKAHEAD = 3       # batches of load prefetch; xt bufs = LOOKAHEAD + 1
N_SUM_ACT = 4       # of the 10 channel-sum chunks, how many go to ACT


def _split_waits(nc: bass.Bass) -> None:
    """This toolchain's walrus accepts only ONE sync-wait per instruction
    (setupSyncWait: 'Too many sync wait commands') while Tile emits several.
    Hoist all-but-one wait onto standalone EventSemaphore instructions
    placed immediately before, on the same engine — semantically identical
    (sequencer stalls at each wait in order)."""
    for f in nc.m.functions:
        for blk in f.blocks:
            insts = list(blk.instructions)
            out, changed = [], False
            for inst in insts:
                si = inst.sync_info
                if si is not None and len(si.on_wait) > 1:
                    waits = list(si.on_wait)
                    for i, w in enumerate(waits[:-1]):
                        ev = mybir.InstEventSemaphore(
                            name=f"{inst.name}-sw{i}", ins=[], outs=[]
                        )
                        ev.engine = inst.engine
                        ev.sync_info = mybir.SyncInfo(on_wait=[w], on_update=[])
                        out.append(ev)
                    si.on_wait = [waits[-1]]
                    changed = True
                out.append(inst)
            if changed:
                blk.instructions = out


def build_program(repeat: int = 1) -> bass.Bass:
    """Emit the single-core SPMD program (same program, per-core data).

    repeat > 1 re-runs the whole batch loop (idempotent) — used only for
    launch-overhead-cancelling timing in test.py.
    """
    nc = bass.Bass()

    # Feature maps, packed on host: xin[p, b, t, j*HW + f] =
    #   T_t[b, 128*j + p, f]  (T_0 = y, T_{1+k} = x_k), fp16.
    d_x = nc.declare_dram_parameter("xin", [128, B_LOC, NT, FD], FP16, isOutput=False)
    d_c1 = nc.declare_dram_parameter("conv1_w", [MID, FEAT], FP32, isOutput=False)
    d_gamma = nc.declare_dram_parameter("bn_gamma", [MID, 1], FP32, isOutput=False)
    d_beta = nc.declare_dram_parameter("bn_beta", [MID, 1], FP32, isOutput=False)
    d_mean = nc.declare_dram_parameter("bn_mean", [MID, 1], FP32, isOutput=False)
    d_var = nc.declare_dram_parameter("bn_var", [MID, 1], FP32, isOutput=False)
    d_c2 = nc.declare_dram_parameter("conv2_w", [NCH, 128, MID], FP32, isOutput=False)
    d_c2b = nc.declare_dram_parameter("conv2_b", [1, NCH, 128], FP32, isOutput=False)
    d_out = nc.declare_dram_parameter("out", [128, B_LOC, FD], FP16, isOutput=True)

    with tile.TileContext(nc) as tc, ExitStack() as ctx:
        cpool = ctx.enter_context(tc.tile_pool(name="cpool", bufs=1))
        ppool = ctx.enter_context(tc.tile_pool(name="ppool", bufs=2, space="PSUM"))
        dpool = ctx.enter_context(tc.tile_pool(name="dpool", bufs=2))
        spool = ctx.enter_context(tc.tile_pool(name="spool", bufs=2))

        # ---------------- parameter prep (once) ----------------
        # Transposed param layouts via DMA-transpose straight from DRAM, then
        # "laundered" through one DVE copy each so PE matmuls (which tolerate
        # only ONE sync-wait on their embedded fp32 weight load) depend on a
        # single producer proc (DVE).
        # conv1_w [32, 1280] -> w1T [128, (j, m)]  (w1T[p, j, m] = conv1_w[m, 128j+p])
        w1s = cpool.tile([128, NCH, MID], FP32, name="w1s", tag="w1s")
        w1T = cpool.tile([128, NCH, MID], FP32, name="w1T", tag="w1T")
        for j in range(NCH):
            nc.sync.dma_start(
                out=w1s[:, j, :],
                in_=d_c1[:, j * 128 : (j + 1) * 128].rearrange("m p -> p m"),
            )
            nc.vector.tensor_copy(w1T[:, j, :], w1s[:, j, :])

        # conv2_w [(j p), m] + conv2_b -> w2T [33, (j, p)]:
        # w2T[m, j, p] = conv2_w[128j+p, m] for m<32; w2T[32, j, p] = conv2_b[128j+p].
        # The gate logits then come out of ONE matmul chain with bias included
        # (h_sb row 32 is pinned to 1).
        w2s = cpool.tile([MID + 1, NCH, 128], FP32, name="w2s", tag="w2s")
        w2T = cpool.tile([MID + 1, NCH, 128], FP32, name="w2T", tag="w2T")
        for j in range(NCH):
            nc.sync.dma_start(out=w2s[:MID, j, :], in_=d_c2[j].rearrange("p m -> m p"))
        nc.sync.dma_start(out=w2s[MID : MID + 1, :, :], in_=d_c2b[:])
        nc.vector.tensor_copy(w2T[:], w2s[:])

        # BN folded affine: h_bn = h_raw * scale_eff + bias_eff, where
        # h_raw = conv1_w @ sums (sums = means * HW), s = gamma/sqrt(var+eps),
        # scale_eff = s / HW, bias_eff = beta - mean * s.
        bn_g = cpool.tile([MID, 1], FP32, name="bn_g", tag="bn_g")
        bn_b = cpool.tile([MID, 1], FP32, name="bn_b", tag="bn_b")
        bn_m = cpool.tile([MID, 1], FP32, name="bn_m", tag="bn_m")
        bn_v = cpool.tile([MID, 1], FP32, name="bn_v", tag="bn_v")
        nc.sync.dma_start(out=bn_g[:], in_=d_gamma[:])
        nc.sync.dma_start(out=bn_b[:], in_=d_beta[:])
        nc.sync.dma_start(out=bn_m[:], in_=d_mean[:])
        nc.sync.dma_start(out=bn_v[:], in_=d_var[:])
        veps = cpool.tile([MID, 1], FP32, name="veps", tag="veps")
        nc.vector.tensor_scalar_add(out=veps[:], in0=bn_v[:], scalar1=float(EPS))
        sq = cpool.tile([MID, 1], FP32, name="sq", tag="sq")
        nc.scalar.sqrt(out=sq[:], in_=veps[:])
        inv = cpool.tile([MID, 1], FP32, name="inv", tag="inv")
        nc.vector.reciprocal(inv[:], sq[:])
        s_bn = cpool.tile([MID, 1], FP32, name="s_bn", tag="s_bn")
        nc.vector.tensor_tensor(out=s_bn[:], in0=bn_g[:], in1=inv[:], op=ALU.mult)
        scale_eff = cpool.tile([MID, 1], FP32, name="scale_eff", tag="scale_eff")
        nc.vector.tensor_scalar_mul(out=scale_eff[:], in0=s_bn[:], scalar1=1.0 / HW)
        ms = cpool.tile([MID, 1], FP32, name="ms", tag="ms")
        nc.vector.tensor_tensor(out=ms[:], in0=bn_m[:], in1=s_bn[:], op=ALU.mult)
        bias_eff = cpool.tile([MID, 1], FP32, name="bias_eff", tag="bias_eff")
        nc.vector.tensor_tensor(out=bias_eff[:], in0=bn_b[:], in1=ms[:], op=ALU.subtract)

        # h_sb: [33, 1] — rows 0..31 get relu(bn(conv1)), row 32 pinned to 1.0
        # so the conv2 matmul adds conv2_b.  bufs=1: the WAR wait this adds
        # (relu(b+1) after conv2 matmuls(b)) is harmless — the MLP chain is
        # far off the DMA roofline.
        h_sb = cpool.tile([MID + 1, 1], FP32, name="h_sb", tag="h_sb")
        nc.vector.memset(h_sb[MID : MID + 1, :], 1.0)

        # ---------------- main loop over local batches ----------------
        bodies = [i % B_LOC for i in range(B_LOC * repeat)]
        xt_q: list = []  # pending loaded tiles, front = next to compute

        def emit_load(b: int):
            xt = dpool.tile([128, NT, FD], FP16, name="xt", tag="xt",
                            bufs=LOOKAHEAD + 1)
            nc.sync.dma_start(out=xt[:], in_=d_x[:, b])
            xt_q.append(xt)

        for idx in range(min(LOOKAHEAD, len(bodies))):
            emit_load(bodies[idx])

        for idx, b in enumerate(bodies):
            if idx + LOOKAHEAD < len(bodies):
                emit_load(bodies[idx + LOOKAHEAD])
            xt = xt_q.pop(0)

            # Channel sums -> mean_t[:, j], j = t*CH + ch (fp32 accumulate).
            mean_t = spool.tile([128, NCH], FP32, name="mean_t", tag="mean_t", bufs=2)
            for j in range(NCH):
                t, ch = divmod(j, CH)
                chunk = xt[:, t, ch * HW : (ch + 1) * HW]
                if j < N_SUM_ACT:
                    scr_a = spool.tile([128, HW], FP16, name="scr_a", tag="scr_a",
                                       bufs=2)
                    nc.scalar.activation(
                        out=scr_a[:], in_=chunk, func=AF.Copy,
                        accum_out=mean_t[:, j : j + 1],
                    )
                else:
                    scr_v = spool.tile([128, HW], FP16, name="scr_v", tag="scr_v",
                                       bufs=2)
                    nc.vector.tensor_scalar(
                        out=scr_v[:], in0=chunk,
                        scalar1=1.0, scalar2=None,
                        op0=ALU.mult, op1=ALU.add,
                        accum_out=mean_t[:, j : j + 1],
                    )

            # Gate MLP on PE: h_raw[mid] = sum_j w1T[:,j,:].T @ sums[:,j]
            hps = ppool.tile([MID, 1], FP32, name="hps", tag="hps")
            for j in range(NCH):
                nc.tensor.matmul(
                    hps[:],
                    w1T[:, j, :],
                    mean_t[:, j : j + 1],
                    start=(j == 0),
                    stop=(j == NCH - 1),
                )
            nc.scalar.activation(
                out=h_sb[:MID, :], in_=hps[:], func=AF.Relu,
                bias=bias_eff[:], scale=scale_eff[:],
            )
            # Logits (bias included via row 32), channel-on-partition:
            # wps[p, j] = w[128j + p]
            wps = ppool.tile([128, NCH], FP32, name="wps", tag="wps")
            for j in range(NCH):
                nc.tensor.matmul(
                    wps[:, j : j + 1], w2T[:, j, :], h_sb[:], start=True, stop=True
                )

            # Gates: cols 0..1 = sigmoid self gate; cols 2..9 = exp for softmax.
            gat = spool.tile([128, NCH], FP32, name="gat", tag="gat", bufs=2)
            nc.scalar.activation(out=gat[:, 0:CH], in_=wps[:, 0:CH], func=AF.Sigmoid)
            nc.scalar.activation(out=gat[:, CH:NCH], in_=wps[:, CH:NCH], func=AF.Exp)
            # softmax over k: columns 2+2k+ch, k=0..3.
            gk = gat[:, CH:NCH].rearrange("p (k c) -> p c k", c=CH)
            esum = spool.tile([128, CH, 1], FP32, name="esum", tag="esum", bufs=2)
            nc.vector.reduce_sum(out=esum[:], in_=gk, axis=mybir.AxisListType.X)
            rinv = spool.tile([128, CH, 1], FP32, name="rinv", tag="rinv", bufs=2)
            nc.vector.reciprocal(rinv[:], esum[:])
            for ch in range(CH):
                nc.vector.tensor_scalar_mul(
                    out=gk[:, ch, :], in0=gk[:, ch, :], scalar1=rinv[:, ch, :]
                )

            # Pass 2: acc = y*w1 + sum_k x_k * g_k, then store.
            acc = dpool.tile([128, FD], FP16, name="acc", tag="acc", bufs=3)
            for ch in range(CH):
                cs = slice(ch * HW, (ch + 1) * HW)
                nc.scalar.activation(
                    out=acc[:, cs], in_=xt[:, 0, cs],
                    func=AF.Copy, scale=gat[:, ch : ch + 1],
                )
            for k in range(K):
                for ch in range(CH):
                    cs = slice(ch * HW, (ch + 1) * HW)
                    nc.vector.scalar_tensor_tensor(
                        out=acc[:, cs],
                        in0=xt[:, 1 + k, cs],
                        scalar=gat[:, CH + CH * k + ch : CH + CH * k + ch + 1],
                        in1=acc[:, cs],
                        op0=ALU.mult,
                        op1=ALU.add,
                    )
            nc.sync.dma_start(out=d_out[:, b], in_=acc[:])

    _split_waits(nc)
    return nc


_CACHE: dict = {}


def _get_program() -> bass.Bass:
    if "nc" not in _CACHE:
        _CACHE["nc"] = build_program()
    return _CACHE["nc"]


def make_in_maps(inputs: dict) -> list:
    """Shard full inputs into per-core input maps (batch-parallel).

    Builds the packed fp16 DRAM image xin[p, b, t, j*HW + f] =
    T_t[b, 128j + p, f] per core.
    """
    f32 = lambda a: np.ascontiguousarray(np.asarray(a), dtype=np.float32)
    names = ["y"] + [f"x{k}" for k in range(K)]
    # [B, NT, CH, 128, HW] -> [128, B, NT, CH*HW], fp16
    allx = np.stack(
        [np.asarray(inputs[nm], dtype=np.float32).reshape(B, CH, 128, HW)
         for nm in names], axis=1,
    )
    allx = np.ascontiguousarray(allx.transpose(3, 0, 1, 2, 4)).astype(np.float16)
    allx = allx.reshape(128, B, NT, FD)

    shared = {
        "conv1_w": f32(inputs["conv1_w"]),
        "bn_gamma": f32(inputs["bn_gamma"]).reshape(MID, 1),
        "bn_beta": f32(inputs["bn_beta"]).reshape(MID, 1),
        "bn_mean": f32(inputs["bn_mean"]).reshape(MID, 1),
        "bn_var": f32(inputs["bn_var"]).reshape(MID, 1),
        "conv2_w": f32(inputs["conv2_w"]).reshape(NCH, 128, MID),
        "conv2_b": f32(inputs["conv2_b"]).reshape(1, NCH, 128),
    }
    in_maps = []
    for core in range(N_CORES):
        sl = slice(core * B_LOC, (core + 1) * B_LOC)
        m = {"xin": np.ascontiguousarray(allx[:, sl])}
        m.update(shared)
        in_maps.append(m)
    return in_maps


def unpack_out(res_core: np.ndarray) -> np.ndarray:
    """[128, B_LOC, FD] fp16 -> [B_LOC, C, H, W] fp32."""
    o = np.asarray(res_core).reshape(128, B_LOC, CH, HW)
    o = o.transpose(1, 2, 0, 3).astype(np.float32)
    return o.reshape(B_LOC, C, H, W)


def kernel(**inputs) -> np.ndarray:
    nc = _get_program()
    in_maps = make_in_maps(inputs)
    res = run_bass_kernel_spmd(nc, in_maps, list(range(N_CORES)))
    _CACHE["last_results"] = res
    out = np.concatenate(
        [unpack_out(res.results[i]["out"]) for i in range(N_CORES)], axis=0
    )
    return out.astype(np.float32)
